# revision 4
# baseline (speedup 1.0000x reference)
"""DBRX-style MoE FFN (16 experts, top-4, SwiGLU) on 8 trn2 NeuronCores.

Strategy (expert-parallel, per sharding hint):
  - Host computes the router (softmax + top-4 + L1 renorm) exactly as the
    reference does (jax on CPU), then dispatches: tokens are gathered per
    expert into fixed-capacity, feature-major (transposed) buffers.
  - Each of the 8 cores owns 2 experts and runs the SwiGLU FFN for the
    tokens routed to them:  Y_e^T = W_d^T @ (silu(W_g^T X^T) * (W_u^T X^T)).
    All matmuls run in bf16 on the tensor engine (fp32 PSUM accumulate).
  - Experts are paired big+small across cores (slot0 = 8 largest, slot1 =
    8 smallest) so the two per-slot capacities C0 >= C1 hug the actual
    token counts, minimizing padded compute while keeping SPMD shapes.
  - Host combines:  out[t] = sum_k top_w[t,k] * Y_{e(t,k)}[slot(t,k)]
    (the "all-reduce" side of the hint, done as part of unsharding).

Everything is feature-major on the device so no transposes are needed:
  gate/up:  lhsT = Wg[dtile, ftile] (K=D on partitions), rhs = X^T[dtile, :]
  down:     lhsT = Wd[ftile, dtile] (K=F on partitions), rhs = H^T[ftile, :]
"""

import math

import ml_dtypes
import numpy as np

D = 2048  # d_model
F = 2048  # ffn hidden
E = 16  # experts
TOPK = 4
P = 128
NCORES = 8
EPC = E // NCORES  # experts per core
DT = D // P  # 16 d-tiles
FT = F // P  # 16 f-tiles
NBLK = 512  # matmul free-dim block (one fp32 PSUM bank)

_BF16 = ml_dtypes.bfloat16

_nc_cache: dict[tuple, object] = {}


def _routing(xt: np.ndarray, w_router: np.ndarray):
    """Router math on CPU via jax, mirroring the reference bit-for-bit."""
    import jax
    import jax.numpy as jnp

    cpu = jax.devices("cpu")[0]
    with jax.default_device(cpu):
        xt_j = jax.device_put(xt, cpu)
        wr_j = jax.device_put(w_router, cpu)
        logits = jnp.einsum("td,ed->te", xt_j, wr_j)
        probs = jax.nn.softmax(logits, axis=-1)
        top_w, top_e = jax.lax.top_k(probs, TOPK)
        top_w = top_w / jnp.sum(jnp.abs(top_w), axis=-1, keepdims=True)
        return np.asarray(top_w), np.asarray(top_e)


def _dispatch(top_e: np.ndarray):
    """Per-expert token lists + the slot of each (token, k) in its expert."""
    T = top_e.shape[0]
    flat_e = top_e.reshape(-1).astype(np.int64)
    flat_t = np.repeat(np.arange(T, dtype=np.int64), TOPK)
    order = np.argsort(flat_e, kind="stable")
    sorted_e = flat_e[order]
    sorted_t = flat_t[order]
    counts = np.bincount(flat_e, minlength=E)
    offsets = np.zeros(E + 1, np.int64)
    np.cumsum(counts, out=offsets[1:])
    slot_sorted = np.arange(T * TOPK, dtype=np.int64) - offsets[sorted_e]
    slot = np.empty(T * TOPK, np.int64)
    slot[order] = slot_sorted
    return counts, offsets, sorted_t, slot.reshape(T, TOPK)


def _cblocks(C: int):
    blocks = []
    c0 = 0
    while c0 < C:
        cn = min(NBLK, C - c0)
        blocks.append((c0, cn))
        c0 += cn
    return blocks


def _build(caps: tuple, repeat: int = 1):
    """Build + compile the per-core Bass program.

    caps = (C0, C1): per-slot token capacities (slot e holds one expert).
    repeat > 1 wraps the whole body in a device-side loop (used only for
    timing: one NEFF execution then runs the body `repeat` times)."""
    key = (tuple(caps), repeat)
    if key in _nc_cache:
        return _nc_cache[key]
    import concourse.bacc as bacc
    import concourse.mybir as mybir
    from concourse import tile

    bf = mybir.dt.bfloat16
    f32 = mybir.dt.float32

    nc = bacc.Bacc(
        "TRN2",
        target_bir_lowering=False,
        debug=False,
        enable_asserts=False,
        num_devices=NCORES,
    )
    xg_t = [
        nc.dram_tensor(f"xg{e}", [DT, P, caps[e]], bf, kind="ExternalInput")
        for e in range(EPC)
    ]
    wg = nc.dram_tensor("wg", [EPC, FT, P, DT, P], bf, kind="ExternalInput")
    wu = nc.dram_tensor("wu", [EPC, FT, P, DT, P], bf, kind="ExternalInput")
    wd = nc.dram_tensor("wd", [EPC, DT, P, FT, P], bf, kind="ExternalInput")
    y_t = [
        nc.dram_tensor(f"y{e}", [DT, P, caps[e]], f32, kind="ExternalOutput")
        for e in range(EPC)
    ]

    silu = mybir.ActivationFunctionType.Silu

    with tile.TileContext(nc) as tc:
        with (
            tc.tile_pool(name="xh", bufs=1) as xh_pool,
            tc.tile_pool(name="wgt", bufs=2) as w_pool,
            tc.tile_pool(name="ev", bufs=4) as ev_pool,
            tc.tile_pool(name="ps", bufs=2, space="PSUM") as ps_pool,
        ):

            def body():
                for e in range(EPC):
                    C = caps[e]
                    blocks = _cblocks(C)
                    xg = xg_t[e]
                    y = y_t[e]
                    x_sb = xh_pool.tile([P, DT, C], bf, tag="x")
                    for dt in range(DT):
                        nc.sync.dma_start(x_sb[:, dt, :], xg[dt])
                    h_sb = xh_pool.tile([P, FT, C], bf, tag="h")

                    # gate + up:  H^T[ft, :] = silu(Wg^T X^T) * (Wu^T X^T)
                    for ft in range(FT):
                        wg_sb = w_pool.tile([P, DT, P], bf, tag="wg")
                        nc.sync.dma_start(wg_sb[:], wg[e, ft])
                        wu_sb = w_pool.tile([P, DT, P], bf, tag="wu")
                        nc.sync.dma_start(wu_sb[:], wu[e, ft])
                        for c0, cn in blocks:
                            pg = ps_pool.tile([P, NBLK], f32, tag="pg")
                            pu = ps_pool.tile([P, NBLK], f32, tag="pu")
                            for dt in range(DT):
                                nc.tensor.matmul(
                                    pg[:, :cn],
                                    wg_sb[:, dt, :],
                                    x_sb[:, dt, c0 : c0 + cn],
                                    start=(dt == 0),
                                    stop=(dt == DT - 1),
                                )
                            for dt in range(DT):
                                nc.tensor.matmul(
                                    pu[:, :cn],
                                    wu_sb[:, dt, :],
                                    x_sb[:, dt, c0 : c0 + cn],
                                    start=(dt == 0),
                                    stop=(dt == DT - 1),
                                )
                            sg = ev_pool.tile([P, NBLK], f32, tag="sg")
                            nc.scalar.activation(sg[:, :cn], pg[:, :cn], silu)
                            nc.vector.tensor_mul(
                                h_sb[:, ft, c0 : c0 + cn], sg[:, :cn], pu[:, :cn]
                            )

                    # down:  Y^T[dt, :] = Wd^T H^T
                    for dt in range(DT):
                        wd_sb = w_pool.tile([P, FT, P], bf, tag="wd")
                        nc.sync.dma_start(wd_sb[:], wd[e, dt])
                        for c0, cn in blocks:
                            py = ps_pool.tile([P, NBLK], f32, tag="py")
                            for ft in range(FT):
                                nc.tensor.matmul(
                                    py[:, :cn],
                                    wd_sb[:, ft, :],
                                    h_sb[:, ft, c0 : c0 + cn],
                                    start=(ft == 0),
                                    stop=(ft == FT - 1),
                                )
                            yo = ev_pool.tile([P, NBLK], f32, tag="yo")
                            nc.vector.tensor_copy(yo[:, :cn], py[:, :cn])
                            nc.sync.dma_start(y[dt, :, c0 : c0 + cn], yo[:, :cn])

            if repeat > 1:
                with tc.For_i(0, repeat, 1):
                    body()
            else:
                body()

    nc.compile()
    _nc_cache[key] = nc
    return nc


def _pack_weight(w: np.ndarray) -> np.ndarray:
    """[E, K, M] fp32 -> [E, MT, P, KT, P] bf16 slabs (lhsT tiles).

    packed[e, mt, p, kt, m] = w[e, kt*P + p, mt*P + m] so that one slab
    packed[e, mt] is a contiguous [P, KT, P] SBUF image.
    """
    Edim, Kdim, Mdim = w.shape
    t = w.astype(_BF16).reshape(Edim, Kdim // P, P, Mdim // P, P)
    return np.ascontiguousarray(t.transpose(0, 3, 2, 1, 4))


def prepare(x, w_router, w_gate, w_up, w_down):
    """Host-side routing + dispatch + packing. Returns everything needed to
    run the device kernel and combine its outputs."""
    x = np.asarray(x, np.float32)
    B, S, _ = x.shape
    T = B * S
    xt = x.reshape(T, D)

    top_w, top_e = _routing(xt, np.asarray(w_router, np.float32))
    counts, offsets, sorted_t, slot = _dispatch(top_e)

    # Pair big+small: slot0 experts = 8 largest, slot1 = 8 smallest.
    # core c runs (desc_order[c], desc_order[15-c]).
    desc = np.argsort(-counts, kind="stable")
    slot_experts = [desc[:NCORES], desc[E - 1 : NCORES - 1 : -1]]  # [slot][core]
    caps = tuple(
        max(P, int(math.ceil(counts[se].max() / P)) * P) for se in slot_experts
    )

    def gather_xt(e_id, C):
        buf = np.zeros((D, C), dtype=_BF16)
        cnt = int(counts[e_id])
        if cnt:
            tok = sorted_t[offsets[e_id] : offsets[e_id] + cnt]
            buf[:, :cnt] = xt[tok].T.astype(_BF16)
        return buf.reshape(DT, P, C)

    wgp = _pack_weight(np.asarray(w_gate, np.float32))
    wup = _pack_weight(np.asarray(w_up, np.float32))
    wdp = _pack_weight(np.asarray(w_down, np.float32))

    in_maps = []
    for c in range(NCORES):
        experts = [int(slot_experts[s][c]) for s in range(EPC)]
        m = {}
        for s in range(EPC):
            m[f"xg{s}"] = gather_xt(experts[s], caps[s])
        m["wg"] = np.ascontiguousarray(wgp[experts])
        m["wu"] = np.ascontiguousarray(wup[experts])
        m["wd"] = np.ascontiguousarray(wdp[experts])
        in_maps.append(m)

    meta = dict(
        B=B,
        S=S,
        T=T,
        caps=caps,
        slot_experts=slot_experts,
        top_w=top_w,
        top_e=top_e,
        slot=slot,
        shape=x.shape,
    )
    return in_maps, meta


def combine(results, meta):
    """out[t] = sum_k top_w[t,k] * Y_{top_e(t,k)}[slot(t,k)]."""
    T = meta["T"]
    caps = meta["caps"]
    slot_experts = meta["slot_experts"]
    # Flat row table: expert e's rows live at base[e] + slot_index.
    total_rows = NCORES * sum(caps)
    rows = np.empty((total_rows, D), np.float32)
    base = np.zeros(E, np.int64)
    r0 = 0
    for s in range(EPC):
        C = caps[s]
        for c in range(NCORES):
            e_id = int(slot_experts[s][c])
            # y{s} from core c: [DT, P, C] = Y^T; transpose to [C, D] rows
            yT = np.asarray(results[c][f"y{s}"]).reshape(D, C)
            rows[r0 : r0 + C] = yT.T
            base[e_id] = r0
            r0 += C
    top_w, top_e, slot = meta["top_w"], meta["top_e"], meta["slot"]
    out = np.zeros((T, D), np.float32)
    for k in range(TOPK):
        idx = base[top_e[:, k]] + slot[:, k]
        out += top_w[:, k, None].astype(np.float32) * rows[idx]
    return out.reshape(meta["shape"]).astype(np.float32)


def kernel(x, w_router, w_gate, w_up, w_down):
    from concourse.bass_utils import run_bass_kernel_spmd

    in_maps, meta = prepare(x, w_router, w_gate, w_up, w_down)
    nc = _build(meta["caps"])
    res = run_bass_kernel_spmd(nc, in_maps, core_ids=list(range(NCORES)))
    return combine(res.results, meta)


# revision 9
# speedup vs baseline: 1.2248x; 1.2248x over previous
"""DBRX-style MoE FFN (16 experts, top-4, SwiGLU) on 8 trn2 NeuronCores.

Strategy (expert-parallel, per sharding hint):
  - Host computes the router (softmax + top-4 + L1 renorm) exactly as the
    reference does (jax on CPU), then dispatches: tokens are gathered per
    expert into fixed-capacity, feature-major (transposed) buffers.
  - Each of the 8 cores owns 2 experts and runs the SwiGLU FFN for the
    tokens routed to them:  Y_e^T = W_d^T @ (silu(W_g^T X^T) * (W_u^T X^T)).
    All matmuls run in bf16 on the tensor engine (fp32 PSUM accumulate).
  - Experts are paired big+small across cores (slot0 = 8 largest, slot1 =
    8 smallest) so the two per-slot capacities C0 >= C1 hug the actual
    token counts, minimizing padded compute while keeping SPMD shapes.
  - Host combines:  out[t] = sum_k top_w[t,k] * Y_{e(t,k)}[slot(t,k)]
    (the "all-reduce" side of the hint, done as part of unsharding).

Everything is feature-major on the device so no transposes are needed:
  gate/up:  lhsT = Wg[dtile, ftile] (K=D on partitions), rhs = X^T[dtile, :]
  down:     lhsT = Wd[ftile, dtile] (K=F on partitions), rhs = H^T[ftile, :]
"""

import math

import ml_dtypes
import numpy as np

D = 2048  # d_model
F = 2048  # ffn hidden
E = 16  # experts
TOPK = 4
P = 128
NCORES = 8
EPC = E // NCORES  # experts per core
DT = D // P  # 16 d-tiles
FT = F // P  # 16 f-tiles
NBLK = 512  # matmul free-dim block (one fp32 PSUM bank)

_BF16 = ml_dtypes.bfloat16

_nc_cache: dict[tuple, object] = {}


def _routing(xt: np.ndarray, w_router: np.ndarray):
    """Router math on CPU via jax, mirroring the reference bit-for-bit."""
    import jax
    import jax.numpy as jnp

    cpu = jax.devices("cpu")[0]
    with jax.default_device(cpu):
        xt_j = jax.device_put(xt, cpu)
        wr_j = jax.device_put(w_router, cpu)
        logits = jnp.einsum("td,ed->te", xt_j, wr_j)
        probs = jax.nn.softmax(logits, axis=-1)
        top_w, top_e = jax.lax.top_k(probs, TOPK)
        top_w = top_w / jnp.sum(jnp.abs(top_w), axis=-1, keepdims=True)
        return np.asarray(top_w), np.asarray(top_e)


def _dispatch(top_e: np.ndarray):
    """Per-expert token lists + the slot of each (token, k) in its expert."""
    T = top_e.shape[0]
    flat_e = top_e.reshape(-1).astype(np.int64)
    flat_t = np.repeat(np.arange(T, dtype=np.int64), TOPK)
    order = np.argsort(flat_e, kind="stable")
    sorted_e = flat_e[order]
    sorted_t = flat_t[order]
    counts = np.bincount(flat_e, minlength=E)
    offsets = np.zeros(E + 1, np.int64)
    np.cumsum(counts, out=offsets[1:])
    slot_sorted = np.arange(T * TOPK, dtype=np.int64) - offsets[sorted_e]
    slot = np.empty(T * TOPK, np.int64)
    slot[order] = slot_sorted
    return counts, offsets, sorted_t, slot.reshape(T, TOPK)


def _cblocks(C: int):
    blocks = []
    c0 = 0
    while c0 < C:
        cn = min(NBLK, C - c0)
        blocks.append((c0, cn))
        c0 += cn
    return blocks


def _build(caps: tuple, repeat: int = 1, w_bufs: int = 3, ps_bufs: int = 3,
           ev_bufs: int = 4, ilv: bool = False, xchunk: bool = False):
    """Build + compile the per-core Bass program.

    caps = (C0, C1): per-slot token capacities (slot e holds one expert).
    repeat > 1 wraps the whole body in a device-side loop (used only for
    timing: one NEFF execution then runs the body `repeat` times)."""
    key = (tuple(caps), repeat, w_bufs, ps_bufs, ev_bufs, ilv, xchunk)
    if key in _nc_cache:
        return _nc_cache[key]
    import concourse.bacc as bacc
    import concourse.mybir as mybir
    from concourse import tile

    bf = mybir.dt.bfloat16
    f32 = mybir.dt.float32

    nc = bacc.Bacc(
        "TRN2",
        target_bir_lowering=False,
        debug=False,
        enable_asserts=False,
        num_devices=NCORES,
    )
    xg_t = [
        nc.dram_tensor(f"xg{e}", [DT, P, caps[e]], bf, kind="ExternalInput")
        for e in range(EPC)
    ]
    wg = nc.dram_tensor("wg", [EPC, FT, P, DT, P], bf, kind="ExternalInput")
    wu = nc.dram_tensor("wu", [EPC, FT, P, DT, P], bf, kind="ExternalInput")
    wd = nc.dram_tensor("wd", [EPC, DT, P, FT, P], bf, kind="ExternalInput")
    y_t = [
        nc.dram_tensor(f"y{e}", [DT, P, caps[e]], f32, kind="ExternalOutput")
        for e in range(EPC)
    ]

    silu = mybir.ActivationFunctionType.Silu

    with tile.TileContext(nc) as tc:
        with (
            tc.tile_pool(name="xh", bufs=1) as xh_pool,
            tc.tile_pool(name="wgt", bufs=w_bufs) as w_pool,
            tc.tile_pool(name="ev", bufs=ev_bufs) as ev_pool,
            tc.tile_pool(name="psg", bufs=ps_bufs, space="PSUM") as ps_pool,
            tc.tile_pool(name="psd", bufs=2, space="PSUM") as psd_pool,
        ):

            def body():
                for e in range(EPC):
                    C = caps[e]
                    blocks = _cblocks(C)
                    xg = xg_t[e]
                    y = y_t[e]
                    x_sb = xh_pool.tile([P, DT, C], bf, tag="x")
                    if xchunk:
                        for c0, cn in blocks:
                            for dt in range(DT):
                                nc.sync.dma_start(
                                    x_sb[:, dt, c0 : c0 + cn],
                                    xg[dt][:, c0 : c0 + cn],
                                )
                    else:
                        for dt in range(DT):
                            nc.sync.dma_start(x_sb[:, dt, :], xg[dt])
                    h_sb = xh_pool.tile([P, FT, C], bf, tag="h")

                    # gate + up:  H^T[ft, :] = silu(Wg^T X^T) * (Wu^T X^T)
                    for ft in range(FT):
                        wg_sb = w_pool.tile([P, DT, P], bf, tag="wg")
                        nc.sync.dma_start(wg_sb[:], wg[e, ft])
                        wu_sb = w_pool.tile([P, DT, P], bf, tag="wu")
                        nc.sync.dma_start(wu_sb[:], wu[e, ft])
                        for c0, cn in blocks:
                            pg = ps_pool.tile([P, NBLK], f32, tag="pg")
                            pu = ps_pool.tile([P, NBLK], f32, tag="pu")
                            if ilv:
                                for dt in range(DT):
                                    nc.tensor.matmul(
                                        pg[:, :cn],
                                        wg_sb[:, dt, :],
                                        x_sb[:, dt, c0 : c0 + cn],
                                        start=(dt == 0),
                                        stop=(dt == DT - 1),
                                    )
                                    nc.tensor.matmul(
                                        pu[:, :cn],
                                        wu_sb[:, dt, :],
                                        x_sb[:, dt, c0 : c0 + cn],
                                        start=(dt == 0),
                                        stop=(dt == DT - 1),
                                    )
                            else:
                                for dt in range(DT):
                                    nc.tensor.matmul(
                                        pg[:, :cn],
                                        wg_sb[:, dt, :],
                                        x_sb[:, dt, c0 : c0 + cn],
                                        start=(dt == 0),
                                        stop=(dt == DT - 1),
                                    )
                                for dt in range(DT):
                                    nc.tensor.matmul(
                                        pu[:, :cn],
                                        wu_sb[:, dt, :],
                                        x_sb[:, dt, c0 : c0 + cn],
                                        start=(dt == 0),
                                        stop=(dt == DT - 1),
                                    )
                            sg = ev_pool.tile([P, NBLK], f32, tag="sg")
                            nc.scalar.activation(sg[:, :cn], pg[:, :cn], silu)
                            nc.vector.tensor_mul(
                                h_sb[:, ft, c0 : c0 + cn], sg[:, :cn], pu[:, :cn]
                            )

                    # down:  Y^T[dt, :] = Wd^T H^T
                    for dt in range(DT):
                        wd_sb = w_pool.tile([P, FT, P], bf, tag="wd")
                        nc.sync.dma_start(wd_sb[:], wd[e, dt])
                        for c0, cn in blocks:
                            py = psd_pool.tile([P, NBLK], f32, tag="py")
                            for ft in range(FT):
                                nc.tensor.matmul(
                                    py[:, :cn],
                                    wd_sb[:, ft, :],
                                    h_sb[:, ft, c0 : c0 + cn],
                                    start=(ft == 0),
                                    stop=(ft == FT - 1),
                                )
                            yo = ev_pool.tile([P, NBLK], f32, tag="yo")
                            nc.vector.tensor_copy(yo[:, :cn], py[:, :cn])
                            nc.sync.dma_start(y[dt, :, c0 : c0 + cn], yo[:, :cn])

            if repeat > 1:
                with tc.For_i(0, repeat, 1):
                    body()
            else:
                body()

    nc.compile()
    _nc_cache[key] = nc
    return nc


def _pack_weight(w: np.ndarray) -> np.ndarray:
    """[E, K, M] fp32 -> [E, MT, P, KT, P] bf16 slabs (lhsT tiles).

    packed[e, mt, p, kt, m] = w[e, kt*P + p, mt*P + m] so that one slab
    packed[e, mt] is a contiguous [P, KT, P] SBUF image.
    """
    Edim, Kdim, Mdim = w.shape
    t = w.astype(_BF16).reshape(Edim, Kdim // P, P, Mdim // P, P)
    return np.ascontiguousarray(t.transpose(0, 3, 2, 1, 4))


def prepare(x, w_router, w_gate, w_up, w_down):
    """Host-side routing + dispatch + packing. Returns everything needed to
    run the device kernel and combine its outputs."""
    x = np.asarray(x, np.float32)
    B, S, _ = x.shape
    T = B * S
    xt = x.reshape(T, D)

    top_w, top_e = _routing(xt, np.asarray(w_router, np.float32))
    counts, offsets, sorted_t, slot = _dispatch(top_e)

    # Pair big+small: slot0 experts = 8 largest, slot1 = 8 smallest.
    # core c runs (desc_order[c], desc_order[15-c]).
    desc = np.argsort(-counts, kind="stable")
    slot_experts = [desc[:NCORES], desc[E - 1 : NCORES - 1 : -1]]  # [slot][core]
    caps = tuple(
        max(P, int(math.ceil(counts[se].max() / P)) * P) for se in slot_experts
    )

    def gather_xt(e_id, C):
        buf = np.zeros((D, C), dtype=_BF16)
        cnt = int(counts[e_id])
        if cnt:
            tok = sorted_t[offsets[e_id] : offsets[e_id] + cnt]
            buf[:, :cnt] = xt[tok].T.astype(_BF16)
        return buf.reshape(DT, P, C)

    wgp = _pack_weight(np.asarray(w_gate, np.float32))
    wup = _pack_weight(np.asarray(w_up, np.float32))
    wdp = _pack_weight(np.asarray(w_down, np.float32))

    in_maps = []
    for c in range(NCORES):
        experts = [int(slot_experts[s][c]) for s in range(EPC)]
        m = {}
        for s in range(EPC):
            m[f"xg{s}"] = gather_xt(experts[s], caps[s])
        m["wg"] = np.ascontiguousarray(wgp[experts])
        m["wu"] = np.ascontiguousarray(wup[experts])
        m["wd"] = np.ascontiguousarray(wdp[experts])
        in_maps.append(m)

    meta = dict(
        B=B,
        S=S,
        T=T,
        caps=caps,
        slot_experts=slot_experts,
        top_w=top_w,
        top_e=top_e,
        slot=slot,
        shape=x.shape,
    )
    return in_maps, meta


def combine(results, meta):
    """out[t] = sum_k top_w[t,k] * Y_{top_e(t,k)}[slot(t,k)]."""
    T = meta["T"]
    caps = meta["caps"]
    slot_experts = meta["slot_experts"]
    # Flat row table: expert e's rows live at base[e] + slot_index.
    total_rows = NCORES * sum(caps)
    rows = np.empty((total_rows, D), np.float32)
    base = np.zeros(E, np.int64)
    r0 = 0
    for s in range(EPC):
        C = caps[s]
        for c in range(NCORES):
            e_id = int(slot_experts[s][c])
            # y{s} from core c: [DT, P, C] = Y^T; transpose to [C, D] rows
            yT = np.asarray(results[c][f"y{s}"]).reshape(D, C)
            rows[r0 : r0 + C] = yT.T
            base[e_id] = r0
            r0 += C
    top_w, top_e, slot = meta["top_w"], meta["top_e"], meta["slot"]
    out = np.zeros((T, D), np.float32)
    for k in range(TOPK):
        idx = base[top_e[:, k]] + slot[:, k]
        out += top_w[:, k, None].astype(np.float32) * rows[idx]
    return out.reshape(meta["shape"]).astype(np.float32)


def kernel(x, w_router, w_gate, w_up, w_down):
    from concourse.bass_utils import run_bass_kernel_spmd

    in_maps, meta = prepare(x, w_router, w_gate, w_up, w_down)
    nc = _build(meta["caps"])
    res = run_bass_kernel_spmd(nc, in_maps, core_ids=list(range(NCORES)))
    return combine(res.results, meta)
